# revision 27
# baseline (speedup 1.0000x reference)
"""MemoAttention Trainium2 kernel — 8-core SPMD.

Reference computation (B=2, S=2048, D=1024, H=16, Dh=64):
    qp = q @ Wq; kp = k @ Wk; vp = v @ Wv          (per batch)
    scores = (qh @ kh^T) * (1/8) * 2*sigmoid(qh . sw_h)   per head
    attn   = softmax(scores); out = attn @ vh
    gate   = sigmoid(out @ Wg + gb); y = (out * gate) @ Wo

Sharding: core c owns heads {2c, 2c+1} for BOTH batches (head-parallel
attention in a transposed [feature, seq] layout), then an 8-way AllToAll
converts to sequence-parallel (core c -> batch c//4, seq slice c%4) for the
gate/out_proj stage.

All device compute in bf16 (fp32 PSUM accumulate).  Host does only layout
prep (transpose / slice / dtype cast) and the final unshard.
"""

import os
import sys

import numpy as np

sys.path.insert(0, "/opt/trn_rl_repo")

import ml_dtypes

import concourse.bacc as bacc
import concourse.bass as bass
import concourse.bass_utils as bass_utils
import concourse.tile as tile
from concourse import mybir

BF16 = ml_dtypes.bfloat16

D_MODEL = 1024
NHEAD = 16
HEAD_DIM = 64
B = 2
S = 2048
N_CORES = 8
GCOLS = 2 * HEAD_DIM        # 128 d_model columns per core (2 heads)
S_CHUNK = 512
N_SC = S // S_CHUNK         # 4
N_TT = S // 128             # 16 t-tiles
N_IC = D_MODEL // 128       # 8 contraction chunks
S_SLICE = 512               # rows per core in stage 2

FP32 = mybir.dt.float32
BF16_T = mybir.dt.bfloat16

TRACE = False
TRACE_KWARGS = {}
LAST_RESULTS = None
DEBUG = False

_CACHE = {}


def _ensure_ntff_hook():
    """The agent image's antenv lacks axon_hooks; synthesize it so
    run_bass_kernel_spmd(trace=True) can NTFF-profile via libaxon_pjrt."""
    import types

    try:
        from antenv import axon_hooks  # noqa: F401
        return
    except ImportError:
        pass
    import antenv

    mod = types.ModuleType("antenv.axon_hooks")
    _state = {"hook": None}
    mod.set_axon_ntff_profile_hook = lambda h: _state.__setitem__("hook", h)
    mod.get_axon_ntff_profile_hook = lambda: _state["hook"]
    sys.modules["antenv.axon_hooks"] = mod
    antenv.axon_hooks = mod
    try:
        from trn_agent_boot.trn_boot import _ntff_profile_via_ctypes

        hook = _ntff_profile_via_ctypes("/opt/axon/libaxon_pjrt.so")
        if hook is not None:
            mod.set_axon_ntff_profile_hook(hook)
    except Exception as e:  # pragma: no cover
        print(f"ntff hook setup failed ({e}); tracing disabled", file=sys.stderr)


def _build_nc():
    nc = bacc.Bacc(
        "TRN2",
        target_bir_lowering=False,
        debug=False,
        enable_asserts=True,
        num_devices=N_CORES,
    )

    # ---- I/O ----
    qT = nc.dram_tensor("qT", [B, D_MODEL, S], BF16_T, kind="ExternalInput").ap()
    kT = nc.dram_tensor("kT", [B, D_MODEL, S], BF16_T, kind="ExternalInput").ap()
    vT = nc.dram_tensor("vT", [B, D_MODEL, S], BF16_T, kind="ExternalInput").ap()
    wq = nc.dram_tensor("wq", [D_MODEL, GCOLS], BF16_T, kind="ExternalInput").ap()
    wk = nc.dram_tensor("wk", [D_MODEL, GCOLS], BF16_T, kind="ExternalInput").ap()
    wv = nc.dram_tensor("wv", [D_MODEL, GCOLS], BF16_T, kind="ExternalInput").ap()
    swb = nc.dram_tensor("swb", [GCOLS, 2], BF16_T, kind="ExternalInput").ap()
    wg = nc.dram_tensor("wg", [D_MODEL, D_MODEL], BF16_T, kind="ExternalInput").ap()
    wo = nc.dram_tensor("wo", [D_MODEL, D_MODEL], BF16_T, kind="ExternalInput").ap()
    gb = nc.dram_tensor("gb", [128, 8], FP32, kind="ExternalInput").ap()
    yT = nc.dram_tensor("yT", [D_MODEL, S_SLICE], FP32, kind="ExternalOutput").ap()
    dbg = {}
    if DEBUG:
        for nm, shp, dt in [
            ("dbg_qhTs", [B, 128, S], BF16_T),
            ("dbg_kpTs", [B, 128, S], BF16_T),
            ("dbg_vp", [B, N_TT, 128, GCOLS], BF16_T),
            ("dbg_rec", [N_SC, 128, S_CHUNK], FP32),
            ("dbg_bc", [B, N_SC, 128, S_CHUNK], BF16_T),
            ("dbg_nrm", [N_CORES, 128, S_CHUNK], BF16_T),
            ("dbg_oT", [8, 128, S_SLICE], BF16_T),
            ("dbg_gt", [8, 128, S_SLICE], BF16_T),
        ]:
            dbg[nm] = nc.dram_tensor(nm, shp, dt, kind="ExternalOutput").ap()

    EXP = mybir.ActivationFunctionType.Exp
    SIG = mybir.ActivationFunctionType.Sigmoid

    with tile.TileContext(nc) as tc:
        # ---------- resident constants / weights ----------
        with tc.tile_pool(name="const", bufs=1) as cpool:
            ones_sb = cpool.tile([128, 1], BF16_T)
            nc.gpsimd.memset(ones_sb[:], 1.0)
            ones64b = cpool.tile([1, 64], BF16_T)
            nc.gpsimd.memset(ones64b[:], 1.0)
            ones64f = cpool.tile([1, 64], FP32)
            nc.gpsimd.memset(ones64f[:], 1.0)
            wq_sb = cpool.tile([128, N_IC, GCOLS], BF16_T)
            nc.sync.dma_start(wq_sb[:], wq.rearrange("(c p) n -> p c n", p=128))
            wk_sb = cpool.tile([128, N_IC, GCOLS], BF16_T)
            nc.sync.dma_start(wk_sb[:], wk.rearrange("(c p) n -> p c n", p=128))
            wv_sb = cpool.tile([128, N_IC, GCOLS], BF16_T)
            nc.sync.dma_start(wv_sb[:], wv.rearrange("(c p) n -> p c n", p=128))
            swb_sb = cpool.tile([128, 2], BF16_T)
            nc.sync.dma_start(swb_sb[:], swb)
            gb_sb = cpool.tile([128, 8], FP32)
            nc.sync.dma_start(gb_sb[:], gb)

            # persistent per-batch activation tensors
            qhTs = [cpool.tile([128, S], BF16_T, name=f"qhTs{b}") for b in range(B)]
            kpTs = [cpool.tile([128, S], BF16_T, name=f"kpTs{b}") for b in range(B)]
            vp_s = [[cpool.tile([128, GCOLS], BF16_T, name=f"vp{b}_{t}")
                     for t in range(N_TT)] for b in range(B)]
            qpT_raw = [cpool.tile([128, S], BF16_T, name=f"qpT_raw{b}")
                       for b in range(B)]

            # ---------- phase 1: projections (transposed layout) ----------
            with tc.tile_pool(name="xT", bufs=8) as xpool, \
                 tc.tile_pool(name="prps", bufs=3, space="PSUM") as prps, \
                 tc.tile_pool(name="scr", bufs=3) as scr:

                for b in range(B):
                    qch = []
                    for i in range(N_IC):
                        t = xpool.tile([128, S], BF16_T, tag="x", name=f"qch{b}_{i}")
                        nc.sync.dma_start(t[:], qT[b, 128 * i:128 * (i + 1), :])
                        qch.append(t)
                    for sc in range(N_SC):
                        ssl = slice(S_CHUNK * sc, S_CHUNK * (sc + 1))
                        ps = prps.tile([128, S_CHUNK], FP32, tag="pp", bufs=3)
                        for i in range(N_IC):
                            nc.tensor.matmul(
                                ps[:], lhsT=wq_sb[:, i, :], rhs=qch[i][:, ssl],
                                start=(i == 0), stop=(i == N_IC - 1))
                        nc.vector.tensor_copy(qpT_raw[b][:, ssl], ps[:])
                        # scale gate c = 0.25*sigmoid(qp . swb) for this chunk,
                        # both heads landed on partition 0 (side-by-side banks)
                        psc = prps.tile([128, 2 * S_CHUNK], FP32, tag="psc", bufs=1)
                        for hh in range(2):
                            nc.tensor.matmul(
                                psc[0:1, S_CHUNK * hh:S_CHUNK * (hh + 1)],
                                lhsT=swb_sb[:, hh:hh + 1],
                                rhs=qpT_raw[b][:, ssl], start=True, stop=True)
                        csb = scr.tile([1, 2 * S_CHUNK], BF16_T, tag="csb")
                        nc.scalar.activation(csb[:], psc[0:1, :], SIG, scale=1.0)
                        # broadcast along partitions via ones outer product
                        bcp = prps.tile([128, S_CHUNK], FP32, tag="bcp", bufs=1)
                        for hh in range(2):
                            # NOTE: start=True clears has_written only for the
                            # partitions the matmul writes -> every partition
                            # range needs its own start/stop
                            nc.tensor.matmul(
                                bcp[64 * hh:64 * (hh + 1), :],
                                lhsT=ones64b[:],
                                rhs=csb[0:1, S_CHUNK * hh:S_CHUNK * (hh + 1)],
                                start=True, stop=True)
                        bc = scr.tile([128, S_CHUNK], BF16_T, tag="bc")
                        # fold the 0.25 = (1/8 softmax) * (2 gate) factor here
                        nc.vector.tensor_scalar_mul(bc[:], bcp[:], 0.25)
                        nc.vector.tensor_mul(qhTs[b][:, ssl], qpT_raw[b][:, ssl], bc[:])
                        if DEBUG:
                            nc.sync.dma_start(dbg["dbg_bc"][b, sc], bc[:])

                for b in range(B):
                    kch = []
                    for i in range(N_IC):
                        t = xpool.tile([128, S], BF16_T, tag="x", name=f"kch{b}_{i}")
                        nc.sync.dma_start(t[:], kT[b, 128 * i:128 * (i + 1), :])
                        kch.append(t)
                    for sc in range(N_SC):
                        ssl = slice(S_CHUNK * sc, S_CHUNK * (sc + 1))
                        ps = prps.tile([128, S_CHUNK], FP32, tag="pp", bufs=3)
                        for i in range(N_IC):
                            nc.tensor.matmul(
                                ps[:], lhsT=wk_sb[:, i, :], rhs=kch[i][:, ssl],
                                start=(i == 0), stop=(i == N_IC - 1))
                        nc.vector.tensor_copy(kpTs[b][:, ssl], ps[:])

                for b in range(B):
                    vch = []
                    for i in range(N_IC):
                        t = xpool.tile([128, S], BF16_T, tag="x", name=f"vch{b}_{i}")
                        nc.sync.dma_start(t[:], vT[b, 128 * i:128 * (i + 1), :])
                        vch.append(t)
                    for tt in range(N_TT):
                        ps = prps.tile([128, GCOLS], FP32, tag="ppv", bufs=2)
                        for i in range(N_IC):
                            nc.tensor.matmul(
                                ps[:], lhsT=vch[i][:, 128 * tt:128 * (tt + 1)],
                                rhs=wv_sb[:, i, :],
                                start=(i == 0), stop=(i == N_IC - 1))
                        nc.vector.tensor_copy(vp_s[b][tt][:], ps[:])
                if DEBUG:
                    for b in range(B):
                        nc.sync.dma_start(dbg["dbg_qhTs"][b], qhTs[b][:])
                        nc.sync.dma_start(dbg["dbg_kpTs"][b], kpTs[b][:])
                        for tt in range(N_TT):
                            nc.sync.dma_start(dbg["dbg_vp"][b, tt], vp_s[b][tt][:])

            # ---------- phase 2: attention ----------
            with tc.tile_pool(name="dram", bufs=1, space="DRAM") as dram:
                a2a_in = dram.tile([N_CORES, 128, S_CHUNK], BF16_T, name="a2a_in")
                a2a_out = dram.tile([N_CORES, 128, S_SLICE], BF16_T, name="a2a_out")

                with tc.tile_pool(name="scps", bufs=2, space="PSUM") as scps, \
                     tc.tile_pool(name="avps", bufs=2, space="PSUM") as avps, \
                     tc.tile_pool(name="dnps", bufs=1, space="PSUM") as dnps, \
                     tc.tile_pool(name="bcps", bufs=1, space="PSUM") as bcps, \
                     tc.tile_pool(name="attnp", bufs=4) as attnp, \
                     tc.tile_pool(name="nrm", bufs=2) as nrm:
                    for sc in range(N_SC):
                        ssl = slice(S_CHUNK * sc, S_CHUNK * (sc + 1))
                        den = dnps.tile([128, S_CHUNK], FP32, tag="den")
                        av = [avps.tile([128, S_CHUNK], FP32, tag="av",
                                        name=f"av{b}_{sc}") for b in range(B)]
                        for b in range(B):
                            for tt in range(N_TT):
                                tsl = slice(128 * tt, 128 * (tt + 1))
                                sps = scps.tile([128, 2 * S_CHUNK], FP32, tag="sc",
                                                name=f"sc{b}_{sc}_{tt}")
                                for hh in range(2):  # row-tiled K=64 pair
                                    rows = slice(64 * hh, 64 * (hh + 1))
                                    nc.tensor.matmul(
                                        sps[:, S_CHUNK * hh:S_CHUNK * (hh + 1)],
                                        lhsT=kpTs[b][rows, tsl],
                                        rhs=qhTs[b][rows, ssl],
                                        start=True, stop=True)
                                at = attnp.tile([128, 2 * S_CHUNK], BF16_T, tag="at",
                                                name=f"at{b}_{sc}_{tt}")
                                nc.scalar.activation(at[:], sps[:], EXP)
                                for hh in range(2):  # attn @ V, col-tiled pair
                                    nc.tensor.matmul(
                                        av[b][64 * hh:64 * (hh + 1), :],
                                        lhsT=vp_s[b][tt][:, 64 * hh:64 * (hh + 1)],
                                        rhs=at[:, S_CHUNK * hh:S_CHUNK * (hh + 1)],
                                        start=(tt == 0), stop=(tt == N_TT - 1),
                                        skip_group_check=True)
                                for hh in range(2):  # denominator, col-tiled x4
                                    r = 32 * (2 * b + hh)
                                    nc.tensor.matmul(
                                        den[r:r + 1, :],
                                        lhsT=ones_sb[:],
                                        rhs=at[:, S_CHUNK * hh:S_CHUNK * (hh + 1)],
                                        start=(tt == 0), stop=(tt == N_TT - 1),
                                        tile_position=(0, r),
                                        skip_group_check=True)
                        # normalize + stage A2A input (shard d = b*4 + sc)
                        rec = nrm.tile([128, S_CHUNK], FP32, tag="rec")
                        nc.vector.reciprocal_approx_fast(rec[:], den[:])
                        if DEBUG:
                            nc.sync.dma_start(dbg["dbg_rec"][sc], rec[:])
                        # gather the 4 valid rows (0/32/64/96) onto partition 0
                        rst = nrm.tile([1, 4 * S_CHUNK], FP32, tag="rst")
                        for hb in range(4):
                            nc.sync.dma_start(
                                rst[0:1, S_CHUNK * hb:S_CHUNK * (hb + 1)],
                                rec[32 * hb:32 * hb + 1, :])
                        for b in range(B):
                            bcn = bcps.tile([128, S_CHUNK], FP32, tag="bcn")
                            for hh in range(2):
                                o = S_CHUNK * (2 * b + hh)
                                nc.tensor.matmul(
                                    bcn[64 * hh:64 * (hh + 1), :],
                                    lhsT=ones64f[:],
                                    rhs=rst[0:1, o:o + S_CHUNK],
                                    start=True, stop=True,
                                    skip_group_check=True)
                            bcs = nrm.tile([128, S_CHUNK], FP32, tag="bcs")
                            nc.vector.tensor_copy(bcs[:], bcn[:])
                            nrm_t = nrm.tile([128, S_CHUNK], BF16_T, tag="nrmt")
                            nc.vector.tensor_mul(nrm_t[:], av[b][:], bcs[:])
                            nc.sync.dma_start(a2a_in[4 * b + sc, :, :], nrm_t[:])
                            if DEBUG:
                                nc.sync.dma_start(dbg["dbg_nrm"][4 * b + sc], nrm_t[:])

                nc.gpsimd.collective_compute(
                    "AllToAll",
                    mybir.AluOpType.bypass,
                    replica_groups=[list(range(N_CORES))],
                    ins=[a2a_in.opt()],
                    outs=[a2a_out.opt()],
                )

                # ---------- phase 3: gate + out_proj (sequence parallel) ----------
                of = a2a_out.rearrange("a b c -> (a b) c")
                with tc.tile_pool(name="s2", bufs=1) as s2, \
                     tc.tile_pool(name="s2ps", bufs=4, space="PSUM") as s2ps, \
                     tc.tile_pool(name="s2scr", bufs=3) as s2scr:
                    wg_sb = s2.tile([128, N_IC, D_MODEL], BF16_T)
                    nc.sync.dma_start(wg_sb[:], wg.rearrange("(c p) n -> p c n", p=128))
                    wo_sb = s2.tile([128, N_IC, D_MODEL], BF16_T)
                    nc.sync.dma_start(wo_sb[:], wo.rearrange("(c p) n -> p c n", p=128))
                    oT = []
                    for k in range(8):
                        t = s2.tile([128, S_SLICE], BF16_T, name=f"oT{k}")
                        nc.sync.dma_start(t[:], of[128 * k:128 * (k + 1), :])
                        if DEBUG:
                            nc.sync.dma_start(dbg["dbg_oT"][k], t[:])
                        oT.append(t)
                    gt = []
                    for ct in range(8):
                        gps = s2ps.tile([128, S_SLICE], FP32, tag="g")
                        for jc in range(N_IC):
                            nc.tensor.matmul(
                                gps[:], lhsT=wg_sb[:, jc, 128 * ct:128 * (ct + 1)],
                                rhs=oT[jc][:],
                                start=(jc == 0), stop=(jc == N_IC - 1))
                        sg = s2scr.tile([128, S_SLICE], BF16_T, tag="sg")
                        nc.scalar.activation(sg[:], gps[:], SIG,
                                             bias=gb_sb[:, ct:ct + 1])
                        g = s2.tile([128, S_SLICE], BF16_T, name=f"gt{ct}")
                        nc.vector.tensor_mul(g[:], oT[ct][:], sg[:])
                        if DEBUG:
                            nc.sync.dma_start(dbg["dbg_gt"][ct], g[:])
                        gt.append(g)
                    for ct in range(8):
                        yps = s2ps.tile([128, S_SLICE], FP32, tag="y")
                        for jc in range(N_IC):
                            nc.tensor.matmul(
                                yps[:], lhsT=wo_sb[:, jc, 128 * ct:128 * (ct + 1)],
                                rhs=gt[jc][:],
                                start=(jc == 0), stop=(jc == N_IC - 1))
                        yo = s2scr.tile([128, S_SLICE], FP32, tag="yo")
                        nc.vector.tensor_copy(yo[:], yps[:])
                        nc.sync.dma_start(yT[128 * ct:128 * (ct + 1), :], yo[:])

    nc.compile()
    return nc


def _shard_inputs(q, k, v, q_proj_weight, k_proj_weight, v_proj_weight,
                  out_proj_weight, gate_weight, gate_bias, scale_weight):
    in_maps = []
    gbh = np.ascontiguousarray(
        gate_bias.astype(np.float32).reshape(8, 128).T)  # [128, 8]
    wg_h = gate_weight.astype(BF16)
    wo_h = out_proj_weight.astype(BF16)
    qT = np.ascontiguousarray(q.transpose(0, 2, 1)).astype(BF16)
    kT = np.ascontiguousarray(k.transpose(0, 2, 1)).astype(BF16)
    vT = np.ascontiguousarray(v.transpose(0, 2, 1)).astype(BF16)
    for c in range(N_CORES):
        cols = slice(GCOLS * c, GCOLS * (c + 1))
        swb = np.zeros((GCOLS, 2), np.float32)
        swb[0:64, 0] = scale_weight[2 * c]
        swb[64:128, 1] = scale_weight[2 * c + 1]
        in_maps.append({
            "qT": qT,
            "kT": kT,
            "vT": vT,
            "wq": np.ascontiguousarray(q_proj_weight[:, cols]).astype(BF16),
            "wk": np.ascontiguousarray(k_proj_weight[:, cols]).astype(BF16),
            "wv": np.ascontiguousarray(v_proj_weight[:, cols]).astype(BF16),
            "swb": swb.astype(BF16),
            "wg": wg_h,
            "wo": wo_h,
            "gb": gbh,
        })
    return in_maps


def kernel(**inputs):
    global LAST_RESULTS
    if "nc" not in _CACHE:
        _CACHE["nc"] = _build_nc()
    nc = _CACHE["nc"]
    if TRACE:
        _ensure_ntff_hook()
    in_maps = _shard_inputs(**{k: np.asarray(v) for k, v in inputs.items()})
    res = bass_utils.run_bass_kernel_spmd(
        nc, in_maps, core_ids=list(range(N_CORES)),
        trace=TRACE, trace_kwargs=TRACE_KWARGS,
    )
    LAST_RESULTS = res
    y = np.zeros((B, S, D_MODEL), np.float32)
    for c in range(N_CORES):
        b, pos = divmod(c, 4)
        y[b, S_SLICE * pos:S_SLICE * (pos + 1), :] = res.results[c]["yT"].T
    return y


if __name__ == "__main__":
    rng = np.random.default_rng(0)
    fake = {
        "q": rng.normal(size=(B, S, D_MODEL)).astype(np.float32),
        "k": rng.normal(size=(B, S, D_MODEL)).astype(np.float32),
        "v": rng.normal(size=(B, S, D_MODEL)).astype(np.float32),
        "q_proj_weight": rng.normal(size=(D_MODEL, D_MODEL)).astype(np.float32) * 0.02,
        "k_proj_weight": rng.normal(size=(D_MODEL, D_MODEL)).astype(np.float32) * 0.02,
        "v_proj_weight": rng.normal(size=(D_MODEL, D_MODEL)).astype(np.float32) * 0.02,
        "out_proj_weight": rng.normal(size=(D_MODEL, D_MODEL)).astype(np.float32) * 0.02,
        "gate_weight": rng.normal(size=(D_MODEL, D_MODEL)).astype(np.float32) * 0.02,
        "gate_bias": rng.normal(size=(D_MODEL,)).astype(np.float32) * 0.02,
        "scale_weight": rng.normal(size=(NHEAD, HEAD_DIM)).astype(np.float32) * 0.02,
    }
    out = kernel(**fake)
    print("ran", out.shape, out.dtype)
